# revision 18
# baseline (speedup 1.0000x reference)
"""Trainium2 Bass kernel for nn_HadamardExpansionV2 (topk_masking).

Reference computation:
  mask  = hard gumbel-softmax over c1=256, for 2*ce rows  -> numerically an
          exact one-hot matrix scaled by w=(1-s)+s (w==1.0 in fp32 for all rows)
  x_i   = einsum('ec,bcl->bel', mask[0], x)   == gather of channels i0[e]
  x_j   = einsum('ec,bcl->bel', mask[1], x)   == gather of channels i1[e]
  xe    = x_i * x_j                            [B, ce, H, W]
  out   = BatchNorm2d(train mode, batch stats over (B,H,W)) * gamma + beta

Strategy (8 NeuronCores, no collectives):
  - Shard the ce=512 expanded channels: core k owns e in [64k, 64k+64).
  - Host computes argmax indices from (logits+gumbel)/tau (exactly matches
    jax: verified min top-2 gap 3.4e-4 >> fp32 eps) and pre-gathers the
    needed channel pairs into a per-core dense tensor xsel [128, B*L]:
    row s<64 -> x[:, i0[e0+s], :], row s>=64 -> x[:, i1[e0+s-64], :].
    BatchNorm stats for a given e are then fully local to one core.
  - Device (identical program on all 8 cores), per group g of 8 e's
    (partition layout p = (e_sub, b), 8*16 = 128):
      DMA  one combined load xio [128, 2L] (xi cols 0:L, xj cols L:2L)
      DVE  scalar_tensor_tensor: prod = (xi*s)*xj (f16) + accum S
      ACT  Square(prod) -> scratch (dead xio half) + accum SS
      PE   matmul with RR^T/N [128,128]: (mean, ssn) replicated per-partition
      DVE  negvar = mean*mean - ssn        (scalar_tensor_tensor)
      ACT  sd = Sqrt(negvar * (-w^2) + eps)
      DVE  rstd = 1/sd ; A = rstd*gw ; Bneg = mean*A - beta
      DVE  tensor_scalar: out = prod*A - Bneg  (f16, 4x mode)
      DMA  out tile -> out[e, b, l]
  - Mask weight w is folded exactly: gw = gamma*w (host), w^2 in the Sqrt
    scale, so the general path costs nothing (w==1.0 for these inputs).
  - Groups are software-pipelined (prefetch depth 2) so the DVE queue never
    head-of-line blocks on the per-group stats chain.

Input gather dtype f16 (~3.6e-4 l2 err) or i8 with per-row scales
(~1.4e-2 l2 err); output f16, host upcasts. The bass program depends only
on shapes -> compiled once and cached.
"""

import os
import sys
from contextlib import ExitStack

import numpy as np

sys.path.insert(0, "/opt/trn_rl_repo")

import concourse.bass as bass  # noqa: E402
import concourse.tile as tile  # noqa: E402
import concourse.mybir as mybir  # noqa: E402
from concourse import bacc  # noqa: E402
from concourse.bass_utils import run_bass_kernel_spmd  # noqa: E402

# Problem shapes (hardcoded per contract)
B, C1, H, W = 16, 256, 56, 56
L = H * W                      # 3136
CE = 512
NCORES = 8
EPC = CE // NCORES             # 64 e-channels per core
NG = 8                         # groups per core
EG = EPC // NG                 # 8 e-channels per group
N = B * L                      # 50176 elements per channel for BN stats
BN_EPS = 1e-5

F32 = mybir.dt.float32
F16 = mybir.dt.float16
I8 = mybir.dt.int8

NCOEF = 4                      # coef cols: -w^2/gw^2, eps/gw^2, beta, sij

# gather dtype: "f16" (~3.6e-4 rel err) or "i8" (per-row scale, ~1.4e-2)
GATHER_DTYPE = os.environ.get("KERNEL_GATHER_DTYPE", "f16")
# output dtype: f16 halves the out-DMA (6.4MB/core); host upcasts to f32.
OUT_DTYPE = os.environ.get("KERNEL_OUT_DTYPE", "f16")

_PROGRAMS = {}  # (gdt, odt) -> compiled program
LAST_RESULT = None  # BassKernelResults of the most recent run (for profiling)


def _build_program(gdt_name, odt_name):
    """Build + compile the (shape-only) bass program shared by all cores."""
    gdt = {"f16": F16, "i8": I8, "f32": F32}[gdt_name]
    odt = F16 if odt_name == "f16" else F32
    nc = bacc.Bacc("TRN2", target_bir_lowering=False, debug=False,
                   num_devices=NCORES)

    xsel_d = nc.dram_tensor("xsel", [128, N], gdt, kind="ExternalInput").ap()
    coef_d = nc.dram_tensor("coef", [128, NCOEF * NG], F32,
                            kind="ExternalInput").ap()
    rr_d = nc.dram_tensor("rr", [128, 128], F32, kind="ExternalInput").ap()
    # e-major output: each group's [128, L] tile lands as one contiguous
    # block; host transposes back to [B, EPC, L].
    out_d = nc.dram_tensor("out", [EPC, B, L], odt, kind="ExternalOutput").ap()

    # combined per-group input view: [g, (e b), m, l]
    # DRAM offset(m,g,e,b,l) = (m*64 + g*8 + e)*N + b*L + l
    xsel_r = xsel_d.rearrange("(m g e) (b l) -> g (e b) m l",
                              m=2, g=NG, b=B)
    # out[(g e), b, l] -> [g, (e b), l]
    out_r = out_d.rearrange("(g e) b l -> g (e b) l", g=NG)

    with tile.TileContext(nc) as tc, ExitStack() as ctx:
        const_pool = ctx.enter_context(tc.tile_pool(name="consts", bufs=1))
        xio_pool = ctx.enter_context(tc.tile_pool(name="xio", bufs=4))
        prod_pool = ctx.enter_context(tc.tile_pool(name="prod", bufs=5))
        sq_pool = ctx.enter_context(tc.tile_pool(name="sq", bufs=2))
        out_pool = ctx.enter_context(tc.tile_pool(name="outs", bufs=3))
        stats_pool = ctx.enter_context(tc.tile_pool(name="stats", bufs=5))
        small_pool = ctx.enter_context(tc.tile_pool(name="smalls", bufs=4))
        psum_pool = ctx.enter_context(
            tc.tile_pool(name="psum", bufs=5, space="PSUM"))

        # constants (coef is tiny and needed by the first STT; rr is loaded
        # after the first gathers so group 0's data is in flight ASAP)
        coef_sb = const_pool.tile([128, NCOEF * NG], F32)
        nc.sync.dma_start(coef_sb[:], coef_d[:])
        rr_sb = const_pool.tile([128, 128], F32)
        eps_t = const_pool.tile([128, 1], F32)
        nc.vector.memset(eps_t[:], float(BN_EPS))

        # per-group state kept across the software pipeline
        xio = [None] * NG
        prod = [None] * NG
        stats = [None] * NG
        agg = [None] * NG
        sm = [None] * NG

        def load(g):
            xio[g] = xio_pool.tile([128, 2 * L], gdt, tag="xio", name=f"xio{g}")
            dst = xio[g][:].rearrange("p (m l) -> p m l", m=2)
            nc.sync.dma_start(dst, xsel_r[g])

        def produce(g):
            # prod = (xi * s) * xj  (s = combined dequant scale; 1.0 for f16)
            prod[g] = prod_pool.tile([128, L], F16, tag="prod", name=f"prod{g}")
            stats[g] = stats_pool.tile([128, 2], F32, tag="stats", name=f"stats{g}")
            if gdt_name == "i8":
                scal = coef_sb[:, NCOEF * g + 3:NCOEF * g + 4]
            else:
                scal = 1.0
            nc.vector.scalar_tensor_tensor(
                out=prod[g][:],
                in0=xio[g][:, 0:L],
                scalar=scal,
                in1=xio[g][:, L:2 * L],
                op0=mybir.AluOpType.mult,
                op1=mybir.AluOpType.mult,
                accum_out=stats[g][:, 0:1],
            )
            # SS: Square(prod) -> own scratch (xio slot frees after prod)
            sq_t = sq_pool.tile([128, L], F16, tag="sq", name=f"sq{g}")
            nc.scalar.activation(
                out=sq_t[:],
                in_=prod[g][:],
                func=mybir.ActivationFunctionType.Square,
                accum_out=stats[g][:, 1:2],
            )
            # (mean, ssn) replicated on every partition of the group
            agg[g] = psum_pool.tile([128, 2], F32, tag="agg", name=f"agg{g}")
            nc.tensor.matmul(agg[g][:], rr_sb[:], stats[g][:],
                             start=True, stop=True)

        def finalize_stats(g):
            # rstd chain folded so recip directly yields A = gw*rstd:
            #   sd' = sqrt(negvar*(-w^2/gw^2) + eps/gw^2) = sd/gw
            sm[g] = small_pool.tile([128, 7], F32, tag="sm", name=f"sm{g}")
            nc.scalar.activation(out=sm[g][:, 5:7], in_=agg[g][:],
                                 func=mybir.ActivationFunctionType.Copy)
            mean = sm[g][:, 5:6]
            ssn = sm[g][:, 6:7]
            negvar = sm[g][:, 0:1]
            sd = sm[g][:, 1:2]
            av = sm[g][:, 2:3]
            bneg = sm[g][:, 3:4]
            nw2g = coef_sb[:, NCOEF * g + 0:NCOEF * g + 1]
            epsg = coef_sb[:, NCOEF * g + 1:NCOEF * g + 2]
            bet = coef_sb[:, NCOEF * g + 2:NCOEF * g + 3]

            # negvar = mean*mean - ssn   (TS: two per-partition scalars)
            nc.vector.tensor_scalar(out=negvar, in0=mean,
                                    scalar1=mean, scalar2=ssn,
                                    op0=mybir.AluOpType.mult,
                                    op1=mybir.AluOpType.subtract)
            nc.scalar.activation(out=sd, in_=negvar,
                                 func=mybir.ActivationFunctionType.Sqrt,
                                 scale=nw2g, bias=epsg)
            nc.vector.reciprocal(av, sd)
            # bneg = mean*A - beta ; out = prod*A - bneg
            nc.vector.tensor_scalar(out=bneg, in0=mean,
                                    scalar1=av, scalar2=bet,
                                    op0=mybir.AluOpType.mult,
                                    op1=mybir.AluOpType.subtract)

        def finalize_norm(g):
            av = sm[g][:, 2:3]
            bneg = sm[g][:, 3:4]
            out_t = out_pool.tile([128, L], odt, tag="outt")
            nc.vector.tensor_scalar(out=out_t[:], in0=prod[g][:],
                                    scalar1=av, scalar2=bneg,
                                    op0=mybir.AluOpType.mult,
                                    op1=mybir.AluOpType.subtract)
            nc.scalar.dma_start(out_r[g], out_t[:])

        # software pipeline with staggered finalize: the tiny stats chain
        # runs at distance 2 and the norm+store at distance 3, so by issue
        # time every dependency is long done (no engine-queue stalls) and the
        # epilogue chains overlap instead of serializing.
        load(0)
        load(1)
        nc.sync.dma_start(rr_sb[:], rr_d[:])
        load(2)
        for g in range(NG):
            if g + 3 < NG:
                load(g + 3)
            if g >= 2:
                finalize_stats(g - 2)
            if g >= 3:
                finalize_norm(g - 3)
            produce(g)
        finalize_stats(NG - 2)
        finalize_stats(NG - 1)
        for g in range(NG - 3, NG):
            finalize_norm(g)

    nc.compile()
    return nc


def _get_program(gdt_name=None, odt_name=None):
    gdt_name = gdt_name or GATHER_DTYPE
    odt_name = odt_name or OUT_DTYPE
    key = (gdt_name, odt_name)
    if key not in _PROGRAMS:
        _PROGRAMS[key] = _build_program(gdt_name, odt_name)
    return _PROGRAMS[key]


def _host_prep(x, logits, gumbel, tau, gamma, beta):
    """Compute mask indices/weights and build per-core inputs."""
    x = np.asarray(x, dtype=np.float32)
    logits = np.asarray(logits, dtype=np.float32)
    gumbel = np.asarray(gumbel, dtype=np.float32)
    tau_f = np.float32(np.asarray(tau))
    gamma = np.asarray(gamma, dtype=np.float32)
    beta = np.asarray(beta, dtype=np.float32)

    # replicate reference softmax/argmax in fp32 (argmax of z == argmax of
    # softmax(z); verified min top-2 gap 3.4e-4 for these inputs)
    z = (logits + gumbel) / tau_f                     # [2, CE, C1] fp32
    idx = z.argmax(axis=-1)                           # [2, CE]
    zm = z.max(axis=-1, keepdims=True)
    ez = np.exp(z - zm, dtype=np.float32)
    soft = ez / ez.sum(axis=-1, keepdims=True, dtype=np.float32)
    s_hot = np.take_along_axis(soft, idx[..., None], axis=-1)[..., 0]
    w = (np.float32(1.0) - s_hot) + s_hot             # [2, CE] (== 1.0 here)
    weff = (w[0] * w[1]).astype(np.float32)           # [CE]

    # channel-major copy of x for fast row gathers: [C1, B*L]
    xt = np.ascontiguousarray(
        x.reshape(B, C1, L).transpose(1, 0, 2)).reshape(C1, N)
    if GATHER_DTYPE == "f16":
        xq = xt.astype(np.float16)
        xscale = np.ones((C1,), dtype=np.float32)
    elif GATHER_DTYPE == "i8":
        xscale = (np.abs(xt).max(axis=1) / np.float32(127.0)).astype(np.float32)
        xq = np.rint(xt / xscale[:, None]).astype(np.int8)
    else:
        xq = xt
        xscale = np.ones((C1,), dtype=np.float32)

    # RR^T/N: block one-hot outer product (partition p in e-block p//B)
    rr = np.zeros((128, 128), dtype=np.float32)
    inv_n = np.float32(1.0) / np.float32(N)
    for es in range(EG):
        rr[es * B:(es + 1) * B, es * B:(es + 1) * B] = inv_n

    in_maps = []
    for k in range(NCORES):
        e0 = k * EPC
        rows = np.concatenate([idx[0, e0:e0 + EPC], idx[1, e0:e0 + EPC]])
        xsel = np.ascontiguousarray(xq[rows])         # [128, N]

        coef = np.zeros((128, NCOEF * NG), dtype=np.float32)
        p = np.arange(128)
        for g in range(NG):
            el = e0 + g * EG + p // B                 # global e per partition
            wv = weff[el]
            gw = gamma[el] * wv
            assert np.all(gw > 0), "sqrt-fold assumes gamma*w > 0"
            coef[:, NCOEF * g + 0] = -(wv * wv) / (gw * gw)
            coef[:, NCOEF * g + 1] = np.float32(BN_EPS) / (gw * gw)
            coef[:, NCOEF * g + 2] = beta[el]
            # combined dequant scale s_i*s_j per partition
            coef[:, NCOEF * g + 3] = (xscale[idx[0, el]] *
                                      xscale[idx[1, el]])

        in_maps.append({
            "xsel": xsel,
            "coef": coef,
            "rr": rr,
        })
    return in_maps


def _install_ntff_shim():
    """The agent image's antenv lacks axon_hooks; recreate it so
    run_bass_kernel_spmd(trace=True) can capture NTFF profiles."""
    import types
    if "antenv.axon_hooks" in sys.modules:
        return
    mod = types.ModuleType("antenv.axon_hooks")
    _hook = [None]
    mod.set_axon_ntff_profile_hook = lambda h: _hook.__setitem__(0, h)
    mod.get_axon_ntff_profile_hook = lambda: _hook[0]
    sys.modules["antenv.axon_hooks"] = mod
    import antenv
    antenv.axon_hooks = mod
    from trn_agent_boot.trn_boot import _ntff_profile_via_ctypes
    mod.set_axon_ntff_profile_hook(
        _ntff_profile_via_ctypes("/opt/axon/libaxon_pjrt.so"))


def kernel(x, logits, gumbel, tau, gamma, beta):
    global LAST_RESULT
    nc = _get_program()
    in_maps = _host_prep(x, logits, gumbel, tau, gamma, beta)

    trace = bool(int(os.environ.get("KERNEL_PROFILE", "0")))
    if trace:
        try:
            _install_ntff_shim()
        except Exception:
            trace = False
    try:
        res = run_bass_kernel_spmd(nc, in_maps, list(range(NCORES)),
                                   trace=trace)
    except Exception:
        if not trace:
            raise
        res = run_bass_kernel_spmd(nc, in_maps, list(range(NCORES)),
                                   trace=False)
    LAST_RESULT = res

    out = np.empty((B, CE, L), dtype=np.float32)
    for k in range(NCORES):
        out[:, k * EPC:(k + 1) * EPC, :] = res.results[k]["out"].transpose(1, 0, 2)
    return out.reshape(B, CE, H, W)


# revision 19
# speedup vs baseline: 1.1233x; 1.1233x over previous
"""Trainium2 Bass kernel for nn_HadamardExpansionV2 (topk_masking).

Reference computation:
  mask  = hard gumbel-softmax over c1=256, for 2*ce rows  -> numerically an
          exact one-hot matrix scaled by w=(1-s)+s (w==1.0 in fp32 for all rows)
  x_i   = einsum('ec,bcl->bel', mask[0], x)   == gather of channels i0[e]
  x_j   = einsum('ec,bcl->bel', mask[1], x)   == gather of channels i1[e]
  xe    = x_i * x_j                            [B, ce, H, W]
  out   = BatchNorm2d(train mode, batch stats over (B,H,W)) * gamma + beta

Strategy (8 NeuronCores, no collectives):
  - Shard the ce=512 expanded channels: core k owns e in [64k, 64k+64).
  - Host computes argmax indices from (logits+gumbel)/tau (exactly matches
    jax: verified min top-2 gap 3.4e-4 >> fp32 eps) and pre-gathers the
    needed channel pairs into a per-core dense tensor xsel [128, B*L]:
    row s<64 -> x[:, i0[e0+s], :], row s>=64 -> x[:, i1[e0+s-64], :].
    BatchNorm stats for a given e are then fully local to one core.
  - Device (identical program on all 8 cores), per group g of 8 e's
    (partition layout p = (e_sub, b), 8*16 = 128):
      DMA  one combined load xio [128, 2L] (xi cols 0:L, xj cols L:2L)
      DVE  scalar_tensor_tensor: prod = (xi*s)*xj (f16) + accum S
      ACT  Square(prod) -> scratch (dead xio half) + accum SS
      PE   matmul with RR^T/N [128,128]: (mean, ssn) replicated per-partition
      DVE  negvar = mean*mean - ssn        (scalar_tensor_tensor)
      ACT  sd = Sqrt(negvar * (-w^2) + eps)
      DVE  rstd = 1/sd ; A = rstd*gw ; Bneg = mean*A - beta
      DVE  tensor_scalar: out = prod*A - Bneg  (f16, 4x mode)
      DMA  out tile -> out[e, b, l]
  - Mask weight w is folded exactly: gw = gamma*w (host), w^2 in the Sqrt
    scale, so the general path costs nothing (w==1.0 for these inputs).
  - Groups are software-pipelined (prefetch depth 2) so the DVE queue never
    head-of-line blocks on the per-group stats chain.

Input gather dtype f16 (~3.6e-4 l2 err) or i8 with per-row scales
(~1.4e-2 l2 err); output f16, host upcasts. The bass program depends only
on shapes -> compiled once and cached.
"""

import os
import sys
from contextlib import ExitStack

import numpy as np

sys.path.insert(0, "/opt/trn_rl_repo")

import concourse.bass as bass  # noqa: E402
import concourse.tile as tile  # noqa: E402
import concourse.mybir as mybir  # noqa: E402
from concourse import bacc  # noqa: E402
from concourse.bass_utils import run_bass_kernel_spmd  # noqa: E402

# Problem shapes (hardcoded per contract)
B, C1, H, W = 16, 256, 56, 56
L = H * W                      # 3136
CE = 512
NCORES = 8
EPC = CE // NCORES             # 64 e-channels per core
NG = 8                         # groups per core
EG = EPC // NG                 # 8 e-channels per group
N = B * L                      # 50176 elements per channel for BN stats
BN_EPS = 1e-5

F32 = mybir.dt.float32
F16 = mybir.dt.float16
I8 = mybir.dt.int8

NCOEF = 4                      # coef cols: -w^2/gw^2, eps/gw^2, beta, sij

# gather dtype: "f16" (~3.6e-4 rel err) or "i8" (per-row scale, ~1.4e-2)
GATHER_DTYPE = os.environ.get("KERNEL_GATHER_DTYPE", "f16")
# output dtype: f16 halves the out-DMA (6.4MB/core); host upcasts to f32.
OUT_DTYPE = os.environ.get("KERNEL_OUT_DTYPE", "f16")

_PROGRAMS = {}  # (gdt, odt) -> compiled program
LAST_RESULT = None  # BassKernelResults of the most recent run (for profiling)


def _build_program(gdt_name, odt_name):
    """Build + compile the (shape-only) bass program shared by all cores."""
    gdt = {"f16": F16, "i8": I8, "f32": F32}[gdt_name]
    odt = F16 if odt_name == "f16" else F32
    nc = bacc.Bacc("TRN2", target_bir_lowering=False, debug=False,
                   num_devices=NCORES)

    xsel_d = nc.dram_tensor("xsel", [128, N], gdt, kind="ExternalInput").ap()
    coef_d = nc.dram_tensor("coef", [128, NCOEF * NG], F32,
                            kind="ExternalInput").ap()
    rr_d = nc.dram_tensor("rr", [128, 128], F32, kind="ExternalInput").ap()
    # e-major output: each group's [128, L] tile lands as one contiguous
    # block; host transposes back to [B, EPC, L].
    out_d = nc.dram_tensor("out", [EPC, B, L], odt, kind="ExternalOutput").ap()

    # combined per-group input view: [g, (e b), m, l]
    # DRAM offset(m,g,e,b,l) = (m*64 + g*8 + e)*N + b*L + l
    xsel_r = xsel_d.rearrange("(m g e) (b l) -> g (e b) m l",
                              m=2, g=NG, b=B)
    # out[(g e), b, l] -> [g, (e b), l]
    out_r = out_d.rearrange("(g e) b l -> g (e b) l", g=NG)

    with tile.TileContext(nc) as tc, ExitStack() as ctx:
        const_pool = ctx.enter_context(tc.tile_pool(name="consts", bufs=1))
        xio_pool = ctx.enter_context(tc.tile_pool(name="xio", bufs=4))
        prod_pool = ctx.enter_context(tc.tile_pool(name="prod", bufs=5))
        sq_pool = ctx.enter_context(tc.tile_pool(name="sq", bufs=2))
        out_pool = ctx.enter_context(tc.tile_pool(name="outs", bufs=5))
        stats_pool = ctx.enter_context(tc.tile_pool(name="stats", bufs=5))
        small_pool = ctx.enter_context(tc.tile_pool(name="smalls", bufs=4))
        psum_pool = ctx.enter_context(
            tc.tile_pool(name="psum", bufs=5, space="PSUM"))

        # constants (coef is tiny and needed by the first STT; rr is loaded
        # after the first gathers so group 0's data is in flight ASAP)
        coef_sb = const_pool.tile([128, NCOEF * NG], F32)
        nc.sync.dma_start(coef_sb[:], coef_d[:])
        rr_sb = const_pool.tile([128, 128], F32)
        eps_t = const_pool.tile([128, 1], F32)
        nc.vector.memset(eps_t[:], float(BN_EPS))

        # per-group state kept across the software pipeline
        xio = [None] * NG
        prod = [None] * NG
        stats = [None] * NG
        agg = [None] * NG
        sm = [None] * NG

        def load(g):
            xio[g] = xio_pool.tile([128, 2 * L], gdt, tag="xio", name=f"xio{g}")
            dst = xio[g][:].rearrange("p (m l) -> p m l", m=2)
            nc.sync.dma_start(dst, xsel_r[g])

        def produce(g):
            # prod = (xi * s) * xj  (s = combined dequant scale; 1.0 for f16)
            prod[g] = prod_pool.tile([128, L], F16, tag="prod", name=f"prod{g}")
            stats[g] = stats_pool.tile([128, 2], F32, tag="stats", name=f"stats{g}")
            if gdt_name == "i8":
                scal = coef_sb[:, NCOEF * g + 3:NCOEF * g + 4]
            else:
                scal = 1.0
            nc.vector.scalar_tensor_tensor(
                out=prod[g][:],
                in0=xio[g][:, 0:L],
                scalar=scal,
                in1=xio[g][:, L:2 * L],
                op0=mybir.AluOpType.mult,
                op1=mybir.AluOpType.mult,
                accum_out=stats[g][:, 0:1],
            )
            # SS: Square(prod) -> own scratch (xio slot frees after prod)
            sq_t = sq_pool.tile([128, L], F16, tag="sq", name=f"sq{g}")
            nc.scalar.activation(
                out=sq_t[:],
                in_=prod[g][:],
                func=mybir.ActivationFunctionType.Square,
                accum_out=stats[g][:, 1:2],
            )
            # (mean, ssn) replicated on every partition of the group
            agg[g] = psum_pool.tile([128, 2], F32, tag="agg", name=f"agg{g}")
            nc.tensor.matmul(agg[g][:], rr_sb[:], stats[g][:],
                             start=True, stop=True)

        def finalize_stats(g):
            # rstd chain folded so recip directly yields A = gw*rstd:
            #   sd' = sqrt(negvar*(-w^2/gw^2) + eps/gw^2) = sd/gw
            sm[g] = small_pool.tile([128, 7], F32, tag="sm", name=f"sm{g}")
            nc.scalar.activation(out=sm[g][:, 5:7], in_=agg[g][:],
                                 func=mybir.ActivationFunctionType.Copy)
            mean = sm[g][:, 5:6]
            ssn = sm[g][:, 6:7]
            negvar = sm[g][:, 0:1]
            sd = sm[g][:, 1:2]
            av = sm[g][:, 2:3]
            bneg = sm[g][:, 3:4]
            nw2g = coef_sb[:, NCOEF * g + 0:NCOEF * g + 1]
            epsg = coef_sb[:, NCOEF * g + 1:NCOEF * g + 2]
            bet = coef_sb[:, NCOEF * g + 2:NCOEF * g + 3]

            # negvar = mean*mean - ssn   (TS: two per-partition scalars)
            nc.vector.tensor_scalar(out=negvar, in0=mean,
                                    scalar1=mean, scalar2=ssn,
                                    op0=mybir.AluOpType.mult,
                                    op1=mybir.AluOpType.subtract)
            nc.scalar.activation(out=sd, in_=negvar,
                                 func=mybir.ActivationFunctionType.Sqrt,
                                 scale=nw2g, bias=epsg)
            nc.vector.reciprocal(av, sd)
            # bneg = mean*A - beta ; out = prod*A - bneg
            nc.vector.tensor_scalar(out=bneg, in0=mean,
                                    scalar1=av, scalar2=bet,
                                    op0=mybir.AluOpType.mult,
                                    op1=mybir.AluOpType.subtract)

        def finalize_norm(g):
            av = sm[g][:, 2:3]
            bneg = sm[g][:, 3:4]
            out_t = out_pool.tile([128, L], odt, tag="outt")
            nc.vector.tensor_scalar(out=out_t[:], in0=prod[g][:],
                                    scalar1=av, scalar2=bneg,
                                    op0=mybir.AluOpType.mult,
                                    op1=mybir.AluOpType.subtract)
            nc.scalar.dma_start(out_r[g], out_t[:])

        # software pipeline with staggered finalize: the tiny stats chain
        # runs at distance 2 and the norm+store at distance 3, so by issue
        # time every dependency is long done (no engine-queue stalls) and the
        # epilogue chains overlap instead of serializing.
        load(0)
        load(1)
        nc.sync.dma_start(rr_sb[:], rr_d[:])
        load(2)
        for g in range(NG):
            if g + 3 < NG:
                load(g + 3)
            if g >= 2:
                finalize_stats(g - 2)
            if g >= 3:
                finalize_norm(g - 3)
            produce(g)
        finalize_stats(NG - 2)
        finalize_stats(NG - 1)
        for g in range(NG - 3, NG):
            finalize_norm(g)

    nc.compile()
    return nc


def _get_program(gdt_name=None, odt_name=None):
    gdt_name = gdt_name or GATHER_DTYPE
    odt_name = odt_name or OUT_DTYPE
    key = (gdt_name, odt_name)
    if key not in _PROGRAMS:
        _PROGRAMS[key] = _build_program(gdt_name, odt_name)
    return _PROGRAMS[key]


def _host_prep(x, logits, gumbel, tau, gamma, beta):
    """Compute mask indices/weights and build per-core inputs."""
    x = np.asarray(x, dtype=np.float32)
    logits = np.asarray(logits, dtype=np.float32)
    gumbel = np.asarray(gumbel, dtype=np.float32)
    tau_f = np.float32(np.asarray(tau))
    gamma = np.asarray(gamma, dtype=np.float32)
    beta = np.asarray(beta, dtype=np.float32)

    # replicate reference softmax/argmax in fp32 (argmax of z == argmax of
    # softmax(z); verified min top-2 gap 3.4e-4 for these inputs)
    z = (logits + gumbel) / tau_f                     # [2, CE, C1] fp32
    idx = z.argmax(axis=-1)                           # [2, CE]
    zm = z.max(axis=-1, keepdims=True)
    ez = np.exp(z - zm, dtype=np.float32)
    soft = ez / ez.sum(axis=-1, keepdims=True, dtype=np.float32)
    s_hot = np.take_along_axis(soft, idx[..., None], axis=-1)[..., 0]
    w = (np.float32(1.0) - s_hot) + s_hot             # [2, CE] (== 1.0 here)
    weff = (w[0] * w[1]).astype(np.float32)           # [CE]

    # channel-major copy of x for fast row gathers: [C1, B*L]
    xt = np.ascontiguousarray(
        x.reshape(B, C1, L).transpose(1, 0, 2)).reshape(C1, N)
    if GATHER_DTYPE == "f16":
        xq = xt.astype(np.float16)
        xscale = np.ones((C1,), dtype=np.float32)
    elif GATHER_DTYPE == "i8":
        xscale = (np.abs(xt).max(axis=1) / np.float32(127.0)).astype(np.float32)
        xq = np.rint(xt / xscale[:, None]).astype(np.int8)
    else:
        xq = xt
        xscale = np.ones((C1,), dtype=np.float32)

    # RR^T/N: block one-hot outer product (partition p in e-block p//B)
    rr = np.zeros((128, 128), dtype=np.float32)
    inv_n = np.float32(1.0) / np.float32(N)
    for es in range(EG):
        rr[es * B:(es + 1) * B, es * B:(es + 1) * B] = inv_n

    in_maps = []
    for k in range(NCORES):
        e0 = k * EPC
        rows = np.concatenate([idx[0, e0:e0 + EPC], idx[1, e0:e0 + EPC]])
        xsel = np.ascontiguousarray(xq[rows])         # [128, N]

        coef = np.zeros((128, NCOEF * NG), dtype=np.float32)
        p = np.arange(128)
        for g in range(NG):
            el = e0 + g * EG + p // B                 # global e per partition
            wv = weff[el]
            gw = gamma[el] * wv
            assert np.all(gw > 0), "sqrt-fold assumes gamma*w > 0"
            coef[:, NCOEF * g + 0] = -(wv * wv) / (gw * gw)
            coef[:, NCOEF * g + 1] = np.float32(BN_EPS) / (gw * gw)
            coef[:, NCOEF * g + 2] = beta[el]
            # combined dequant scale s_i*s_j per partition
            coef[:, NCOEF * g + 3] = (xscale[idx[0, el]] *
                                      xscale[idx[1, el]])

        in_maps.append({
            "xsel": xsel,
            "coef": coef,
            "rr": rr,
        })
    return in_maps


def _install_ntff_shim():
    """The agent image's antenv lacks axon_hooks; recreate it so
    run_bass_kernel_spmd(trace=True) can capture NTFF profiles."""
    import types
    if "antenv.axon_hooks" in sys.modules:
        return
    mod = types.ModuleType("antenv.axon_hooks")
    _hook = [None]
    mod.set_axon_ntff_profile_hook = lambda h: _hook.__setitem__(0, h)
    mod.get_axon_ntff_profile_hook = lambda: _hook[0]
    sys.modules["antenv.axon_hooks"] = mod
    import antenv
    antenv.axon_hooks = mod
    from trn_agent_boot.trn_boot import _ntff_profile_via_ctypes
    mod.set_axon_ntff_profile_hook(
        _ntff_profile_via_ctypes("/opt/axon/libaxon_pjrt.so"))


def kernel(x, logits, gumbel, tau, gamma, beta):
    global LAST_RESULT
    nc = _get_program()
    in_maps = _host_prep(x, logits, gumbel, tau, gamma, beta)

    trace = bool(int(os.environ.get("KERNEL_PROFILE", "0")))
    if trace:
        try:
            _install_ntff_shim()
        except Exception:
            trace = False
    try:
        res = run_bass_kernel_spmd(nc, in_maps, list(range(NCORES)),
                                   trace=trace)
    except Exception:
        if not trace:
            raise
        res = run_bass_kernel_spmd(nc, in_maps, list(range(NCORES)),
                                   trace=False)
    LAST_RESULT = res

    out = np.empty((B, CE, L), dtype=np.float32)
    for k in range(NCORES):
        out[:, k * EPC:(k + 1) * EPC, :] = res.results[k]["out"].transpose(1, 0, 2)
    return out.reshape(B, CE, H, W)


# revision 20
# speedup vs baseline: 1.1643x; 1.0364x over previous
"""Trainium2 Bass kernel for nn_HadamardExpansionV2 (topk_masking).

Reference computation:
  mask  = hard gumbel-softmax over c1=256, for 2*ce rows  -> numerically an
          exact one-hot matrix scaled by w=(1-s)+s (w==1.0 in fp32 for all rows)
  x_i   = einsum('ec,bcl->bel', mask[0], x)   == gather of channels i0[e]
  x_j   = einsum('ec,bcl->bel', mask[1], x)   == gather of channels i1[e]
  xe    = x_i * x_j                            [B, ce, H, W]
  out   = BatchNorm2d(train mode, batch stats over (B,H,W)) * gamma + beta

Strategy (8 NeuronCores, no collectives):
  - Shard the ce=512 expanded channels: core k owns e in [64k, 64k+64).
  - Host computes argmax indices from (logits+gumbel)/tau (exactly matches
    jax: verified min top-2 gap 3.4e-4 >> fp32 eps) and pre-gathers the
    needed channel pairs into a per-core dense tensor xsel [128, B*L]:
    row s<64 -> x[:, i0[e0+s], :], row s>=64 -> x[:, i1[e0+s-64], :].
    BatchNorm stats for a given e are then fully local to one core.
  - Device (identical program on all 8 cores), per group g of 8 e's
    (partition layout p = (e_sub, b), 8*16 = 128):
      DMA  one combined load xio [128, 2L] (xi cols 0:L, xj cols L:2L)
      DVE  scalar_tensor_tensor: prod = (xi*s)*xj (f16) + accum S
      ACT  Square(prod) -> scratch (dead xio half) + accum SS
      PE   matmul with RR^T/N [128,128]: (mean, ssn) replicated per-partition
      DVE  negvar = mean*mean - ssn        (scalar_tensor_tensor)
      ACT  sd = Sqrt(negvar * (-w^2) + eps)
      DVE  rstd = 1/sd ; A = rstd*gw ; Bneg = mean*A - beta
      DVE  tensor_scalar: out = prod*A - Bneg  (f16, 4x mode)
      DMA  out tile -> out[e, b, l]
  - Mask weight w is folded exactly: gw = gamma*w (host), w^2 in the Sqrt
    scale, so the general path costs nothing (w==1.0 for these inputs).
  - Groups are software-pipelined (prefetch depth 2) so the DVE queue never
    head-of-line blocks on the per-group stats chain.

Input gather dtype f16 (~3.6e-4 l2 err) or i8 with per-row scales
(~1.4e-2 l2 err); output f16, host upcasts. The bass program depends only
on shapes -> compiled once and cached.
"""

import os
import sys
from contextlib import ExitStack

import numpy as np

sys.path.insert(0, "/opt/trn_rl_repo")

import concourse.bass as bass  # noqa: E402
import concourse.tile as tile  # noqa: E402
import concourse.mybir as mybir  # noqa: E402
from concourse import bacc  # noqa: E402
from concourse.bass_utils import run_bass_kernel_spmd  # noqa: E402

# Problem shapes (hardcoded per contract)
B, C1, H, W = 16, 256, 56, 56
L = H * W                      # 3136
CE = 512
NCORES = 8
EPC = CE // NCORES             # 64 e-channels per core
NG = 8                         # groups per core
EG = EPC // NG                 # 8 e-channels per group
N = B * L                      # 50176 elements per channel for BN stats
BN_EPS = 1e-5

F32 = mybir.dt.float32
F16 = mybir.dt.float16
I8 = mybir.dt.int8

NCOEF = 4                      # coef cols: -w^2/gw^2, eps/gw^2, beta, sij

# gather dtype: "f16" (~3.6e-4 rel err) or "i8" (per-row scale, ~1.4e-2)
GATHER_DTYPE = os.environ.get("KERNEL_GATHER_DTYPE", "f16")
# output dtype: f16 halves the out-DMA (6.4MB/core); host upcasts to f32.
OUT_DTYPE = os.environ.get("KERNEL_OUT_DTYPE", "f16")

_PROGRAMS = {}  # (gdt, odt) -> compiled program
LAST_RESULT = None  # BassKernelResults of the most recent run (for profiling)


def _build_program(gdt_name, odt_name):
    """Build + compile the (shape-only) bass program shared by all cores."""
    gdt = {"f16": F16, "i8": I8, "f32": F32}[gdt_name]
    odt = F16 if odt_name == "f16" else F32
    nc = bacc.Bacc("TRN2", target_bir_lowering=False, debug=False,
                   num_devices=NCORES)

    xsel_d = nc.dram_tensor("xsel", [128, N], gdt, kind="ExternalInput").ap()
    coef_d = nc.dram_tensor("coef", [128, NCOEF * NG], F32,
                            kind="ExternalInput").ap()
    rr_d = nc.dram_tensor("rr", [128, 128], F32, kind="ExternalInput").ap()
    # e-major output: each group's [128, L] tile lands as one contiguous
    # block; host transposes back to [B, EPC, L].
    out_d = nc.dram_tensor("out", [EPC, B, L], odt, kind="ExternalOutput").ap()

    # combined per-group input view: [g, (e b), m, l]
    # DRAM offset(m,g,e,b,l) = (m*64 + g*8 + e)*N + b*L + l
    xsel_r = xsel_d.rearrange("(m g e) (b l) -> g (e b) m l",
                              m=2, g=NG, b=B)
    # out[(g e), b, l] -> [g, (e b), l]
    out_r = out_d.rearrange("(g e) b l -> g (e b) l", g=NG)

    with tile.TileContext(nc) as tc, ExitStack() as ctx:
        const_pool = ctx.enter_context(tc.tile_pool(name="consts", bufs=1))
        xio_pool = ctx.enter_context(tc.tile_pool(name="xio", bufs=4))
        prod_pool = ctx.enter_context(tc.tile_pool(name="prod", bufs=5))
        sq_pool = ctx.enter_context(tc.tile_pool(name="sq", bufs=2))
        out_pool = ctx.enter_context(tc.tile_pool(name="outs", bufs=5))
        stats_pool = ctx.enter_context(tc.tile_pool(name="stats", bufs=5))
        small_pool = ctx.enter_context(tc.tile_pool(name="smalls", bufs=4))
        psum_pool = ctx.enter_context(
            tc.tile_pool(name="psum", bufs=5, space="PSUM"))

        # constants (coef is tiny and needed by the first STT; rr is loaded
        # after the first gathers so group 0's data is in flight ASAP)
        coef_sb = const_pool.tile([128, NCOEF * NG], F32)
        nc.scalar.dma_start(coef_sb[:], coef_d[:])
        rr_sb = const_pool.tile([128, 128], F32)
        eps_t = const_pool.tile([128, 1], F32)
        nc.vector.memset(eps_t[:], float(BN_EPS))

        # per-group state kept across the software pipeline
        xio = [None] * NG
        prod = [None] * NG
        stats = [None] * NG
        agg = [None] * NG
        sm = [None] * NG

        def load(g):
            xio[g] = xio_pool.tile([128, 2 * L], gdt, tag="xio", name=f"xio{g}")
            dst = xio[g][:].rearrange("p (m l) -> p m l", m=2)
            nc.sync.dma_start(dst, xsel_r[g])

        def produce(g):
            # prod = (xi * s) * xj  (s = combined dequant scale; 1.0 for f16)
            prod[g] = prod_pool.tile([128, L], F16, tag="prod", name=f"prod{g}")
            stats[g] = stats_pool.tile([128, 2], F32, tag="stats", name=f"stats{g}")
            if gdt_name == "i8":
                scal = coef_sb[:, NCOEF * g + 3:NCOEF * g + 4]
            else:
                scal = 1.0
            nc.vector.scalar_tensor_tensor(
                out=prod[g][:],
                in0=xio[g][:, 0:L],
                scalar=scal,
                in1=xio[g][:, L:2 * L],
                op0=mybir.AluOpType.mult,
                op1=mybir.AluOpType.mult,
                accum_out=stats[g][:, 0:1],
            )
            # SS: Square(prod) -> own scratch (xio slot frees after prod)
            sq_t = sq_pool.tile([128, L], F16, tag="sq", name=f"sq{g}")
            nc.scalar.activation(
                out=sq_t[:],
                in_=prod[g][:],
                func=mybir.ActivationFunctionType.Square,
                accum_out=stats[g][:, 1:2],
            )
            # (mean, ssn) replicated on every partition of the group
            agg[g] = psum_pool.tile([128, 2], F32, tag="agg", name=f"agg{g}")
            nc.tensor.matmul(agg[g][:], rr_sb[:], stats[g][:],
                             start=True, stop=True)

        def stats_a(g):
            # rstd chain folded so recip directly yields A = gw*rstd:
            #   sd' = sqrt(negvar*(-w^2/gw^2) + eps/gw^2) = sd/gw
            sm[g] = small_pool.tile([128, 7], F32, tag="sm", name=f"sm{g}")
            nc.scalar.activation(out=sm[g][:, 5:7], in_=agg[g][:],
                                 func=mybir.ActivationFunctionType.Copy)
            mean = sm[g][:, 5:6]
            ssn = sm[g][:, 6:7]
            negvar = sm[g][:, 0:1]
            # negvar = mean*mean - ssn   (TS: two per-partition scalars)
            nc.vector.tensor_scalar(out=negvar, in0=mean,
                                    scalar1=mean, scalar2=ssn,
                                    op0=mybir.AluOpType.mult,
                                    op1=mybir.AluOpType.subtract)
            nc.scalar.activation(out=sm[g][:, 1:2], in_=negvar,
                                 func=mybir.ActivationFunctionType.Sqrt,
                                 scale=coef_sb[:, NCOEF * g + 0:NCOEF * g + 1],
                                 bias=coef_sb[:, NCOEF * g + 1:NCOEF * g + 2])

        def stats_b(g):
            mean = sm[g][:, 5:6]
            sd = sm[g][:, 1:2]
            av = sm[g][:, 2:3]
            bneg = sm[g][:, 3:4]
            bet = coef_sb[:, NCOEF * g + 2:NCOEF * g + 3]
            nc.vector.reciprocal(av, sd)
            # bneg = mean*A - beta ; out = prod*A - bneg
            nc.vector.tensor_scalar(out=bneg, in0=mean,
                                    scalar1=av, scalar2=bet,
                                    op0=mybir.AluOpType.mult,
                                    op1=mybir.AluOpType.subtract)

        def finalize_norm(g):
            av = sm[g][:, 2:3]
            bneg = sm[g][:, 3:4]
            out_t = out_pool.tile([128, L], odt, tag="outt")
            nc.vector.tensor_scalar(out=out_t[:], in0=prod[g][:],
                                    scalar1=av, scalar2=bneg,
                                    op0=mybir.AluOpType.mult,
                                    op1=mybir.AluOpType.subtract)
            nc.scalar.dma_start(out_r[g], out_t[:])

        # software pipeline: stats chain at distance 2, norm+store at
        # distance 3, with the big norm TS issued BETWEEN negvar and recip so
        # the DVE never idles while ACT runs the Sqrt (chain ping-pong is
        # hidden under useful DVE work). Loads prefetch 3 groups ahead.
        load(0)
        load(1)
        nc.sync.dma_start(rr_sb[:], rr_d[:])
        load(2)
        for g in range(NG + 2):
            if g + 3 < NG:
                load(g + 3)
            if g >= 2:
                stats_a(g - 2)
            if g >= 3:
                finalize_norm(g - 3)
            if g >= 2:
                stats_b(g - 2)
            if g < NG:
                produce(g)
        finalize_norm(NG - 1)

    nc.compile()
    return nc


def _get_program(gdt_name=None, odt_name=None):
    gdt_name = gdt_name or GATHER_DTYPE
    odt_name = odt_name or OUT_DTYPE
    key = (gdt_name, odt_name)
    if key not in _PROGRAMS:
        _PROGRAMS[key] = _build_program(gdt_name, odt_name)
    return _PROGRAMS[key]


def _host_prep(x, logits, gumbel, tau, gamma, beta):
    """Compute mask indices/weights and build per-core inputs."""
    x = np.asarray(x, dtype=np.float32)
    logits = np.asarray(logits, dtype=np.float32)
    gumbel = np.asarray(gumbel, dtype=np.float32)
    tau_f = np.float32(np.asarray(tau))
    gamma = np.asarray(gamma, dtype=np.float32)
    beta = np.asarray(beta, dtype=np.float32)

    # replicate reference softmax/argmax in fp32 (argmax of z == argmax of
    # softmax(z); verified min top-2 gap 3.4e-4 for these inputs)
    z = (logits + gumbel) / tau_f                     # [2, CE, C1] fp32
    idx = z.argmax(axis=-1)                           # [2, CE]
    zm = z.max(axis=-1, keepdims=True)
    ez = np.exp(z - zm, dtype=np.float32)
    soft = ez / ez.sum(axis=-1, keepdims=True, dtype=np.float32)
    s_hot = np.take_along_axis(soft, idx[..., None], axis=-1)[..., 0]
    w = (np.float32(1.0) - s_hot) + s_hot             # [2, CE] (== 1.0 here)
    weff = (w[0] * w[1]).astype(np.float32)           # [CE]

    # channel-major copy of x for fast row gathers: [C1, B*L]
    xt = np.ascontiguousarray(
        x.reshape(B, C1, L).transpose(1, 0, 2)).reshape(C1, N)
    if GATHER_DTYPE == "f16":
        xq = xt.astype(np.float16)
        xscale = np.ones((C1,), dtype=np.float32)
    elif GATHER_DTYPE == "i8":
        xscale = (np.abs(xt).max(axis=1) / np.float32(127.0)).astype(np.float32)
        xq = np.rint(xt / xscale[:, None]).astype(np.int8)
    else:
        xq = xt
        xscale = np.ones((C1,), dtype=np.float32)

    # RR^T/N: block one-hot outer product (partition p in e-block p//B)
    rr = np.zeros((128, 128), dtype=np.float32)
    inv_n = np.float32(1.0) / np.float32(N)
    for es in range(EG):
        rr[es * B:(es + 1) * B, es * B:(es + 1) * B] = inv_n

    in_maps = []
    for k in range(NCORES):
        e0 = k * EPC
        rows = np.concatenate([idx[0, e0:e0 + EPC], idx[1, e0:e0 + EPC]])
        xsel = np.ascontiguousarray(xq[rows])         # [128, N]

        coef = np.zeros((128, NCOEF * NG), dtype=np.float32)
        p = np.arange(128)
        for g in range(NG):
            el = e0 + g * EG + p // B                 # global e per partition
            wv = weff[el]
            gw = gamma[el] * wv
            assert np.all(gw > 0), "sqrt-fold assumes gamma*w > 0"
            coef[:, NCOEF * g + 0] = -(wv * wv) / (gw * gw)
            coef[:, NCOEF * g + 1] = np.float32(BN_EPS) / (gw * gw)
            coef[:, NCOEF * g + 2] = beta[el]
            # combined dequant scale s_i*s_j per partition
            coef[:, NCOEF * g + 3] = (xscale[idx[0, el]] *
                                      xscale[idx[1, el]])

        in_maps.append({
            "xsel": xsel,
            "coef": coef,
            "rr": rr,
        })
    return in_maps


def _install_ntff_shim():
    """The agent image's antenv lacks axon_hooks; recreate it so
    run_bass_kernel_spmd(trace=True) can capture NTFF profiles."""
    import types
    if "antenv.axon_hooks" in sys.modules:
        return
    mod = types.ModuleType("antenv.axon_hooks")
    _hook = [None]
    mod.set_axon_ntff_profile_hook = lambda h: _hook.__setitem__(0, h)
    mod.get_axon_ntff_profile_hook = lambda: _hook[0]
    sys.modules["antenv.axon_hooks"] = mod
    import antenv
    antenv.axon_hooks = mod
    from trn_agent_boot.trn_boot import _ntff_profile_via_ctypes
    mod.set_axon_ntff_profile_hook(
        _ntff_profile_via_ctypes("/opt/axon/libaxon_pjrt.so"))


def kernel(x, logits, gumbel, tau, gamma, beta):
    global LAST_RESULT
    nc = _get_program()
    in_maps = _host_prep(x, logits, gumbel, tau, gamma, beta)

    trace = bool(int(os.environ.get("KERNEL_PROFILE", "0")))
    if trace:
        try:
            _install_ntff_shim()
        except Exception:
            trace = False
    try:
        res = run_bass_kernel_spmd(nc, in_maps, list(range(NCORES)),
                                   trace=trace)
    except Exception:
        if not trace:
            raise
        res = run_bass_kernel_spmd(nc, in_maps, list(range(NCORES)),
                                   trace=False)
    LAST_RESULT = res

    out = np.empty((B, CE, L), dtype=np.float32)
    for k in range(NCORES):
        out[:, k * EPC:(k + 1) * EPC, :] = res.results[k]["out"].transpose(1, 0, 2)
    return out.reshape(B, CE, H, W)
